# revision 27
# baseline (speedup 1.0000x reference)
"""MiniMax sparse-MoE block on 8 Trainium2 NeuronCores.

Strategy (expert-parallel, per the sharding hint):
  - Router (gates matmul + sigmoid + top-2 + weight normalization) runs on
    host CPU with exactly the reference's jax ops, bit-matching its
    routing decisions.  This *is* the dispatch step: tokens are gathered
    per selected expert while building the per-core input shards.
  - Each of the 8 cores owns E/8 = 2 experts.  A core runs the SwitchGLU
    MLP (silu(x@w_gate) * (x@w_up)) @ w_down for the tokens routed to its
    experts only (capacity = max expert load over cores per slot), with
    weights stationary on the PE array and tokens as the moving operand
    (activations kept transposed: [H, tokens]).
  - Matmuls run in fp16 (half the HBM traffic of fp32, full-rate PE);
    PSUM accumulation is fp32.
  - Schedule: ALL weight tiles are SBUF-resident (12.6 MB of 24 MB), so
    every weight DMA is issued up-front in consumption-priority order on
    the two HWDGE queues (sync + scalar) with small leading chunks; the
    PE never waits on buffer reuse and starts ~2 us into the body.
  - y is written back as fp16 (halves the writeback bytes); the host
    combines y[t] = sum over the token's 2 experts of sel * expert_out in
    fp32.
"""

import os
import sys
import functools

for _p in ("/opt/trn_rl_repo", "/root/.axon_site/_ro/trn_rl_repo"):
    if os.path.isdir(_p) and _p not in sys.path:
        sys.path.append(_p)

import numpy as np

T, H, F, E, KTOP = 2048, 1024, 1024, 16, 2
NCORES = 8
EPC = E // NCORES  # experts per core
P = 128
KO = H // P  # contraction chunks per 1024-dim
FB = F // P  # 128-blocks of F
HB = H // P  # 128-blocks of H

MM_MODE = os.environ.get("MOE_MM_MODE", "f16")
CAP_ALIGN = int(os.environ.get("MOE_CAP_ALIGN", "16"))

LAST_RESULTS = None  # BassKernelResults of the most recent device run


def _chunks(cap):
    """Split cap into moving-dim chunks <= 512 (PSUM bank fp32 limit)."""
    out, rem, n = [], cap, -(-cap // 512)
    for i in range(n):
        c = min(512, rem, -(-rem // ((n - i) * 64)) * 64)
        out.append(c)
        rem -= c
    assert sum(out) == cap and all(0 < c <= 512 for c in out), (cap, out)
    return out


@functools.lru_cache(maxsize=4)
def _build_program(caps):
    import concourse.mybir as mybir
    import concourse.tile as tile
    from concourse import bacc

    f32 = mybir.dt.float32
    f16 = mybir.dt.float16
    mm_dt = {"f16": f16, "f32r": mybir.dt.float32r, "f32": f32}[MM_MODE]
    silu = mybir.ActivationFunctionType.Silu

    nc = bacc.Bacc("TRN2", target_bir_lowering=False, debug=False,
                   num_devices=NCORES)

    xt_d, wg_d, wu_d, wd_d, yt_d = [], [], [], [], []
    for s in range(EPC):
        cap = caps[s]
        xt_d.append(nc.dram_tensor(f"xt{s}", [P, KO, cap], mm_dt,
                                   kind="ExternalInput").ap())
        wg_d.append(nc.dram_tensor(f"wg{s}", [P, FB, H], mm_dt,
                                   kind="ExternalInput").ap())
        wu_d.append(nc.dram_tensor(f"wu{s}", [P, FB, H], mm_dt,
                                   kind="ExternalInput").ap())
        wd_d.append(nc.dram_tensor(f"wd{s}", [P, HB, F], mm_dt,
                                   kind="ExternalInput").ap())
        yt_d.append(nc.dram_tensor(f"yt{s}", [HB, P, cap], mm_dt,
                                   kind="ExternalOutput").ap())

    def mm(ps, lhsT, rhs, start, stop):
        nc.tensor.matmul(ps, lhsT=lhsT, rhs=rhs, start=start, stop=stop)

    with tile.TileContext(nc) as tc:
        with (
            tc.tile_pool(name="xp", bufs=1) as xp,
            tc.tile_pool(name="wp", bufs=1) as wp,
            tc.tile_pool(name="sp", bufs=3) as sp,
            tc.tile_pool(name="hp", bufs=1) as hp,
            tc.tile_pool(name="op", bufs=8) as op,
            tc.tile_pool(name="pp", bufs=8, space="PSUM") as pp,
        ):
            # ---- resident tiles, one per DMA chunk ------------------
            # Distinct tags per chunk keep Tile's DMA->matmul deps
            # fine-grained.  Fine (2-f-block) chunks where the PE is
            # close on the stream's heels (slot-0 g/u); coarse (4-f)
            # chunks later, to bound the semaphore count (the exit
            # epilogue clears each semaphore individually).
            xt_t = [xp.tile([P, KO, caps[s]], mm_dt, tag=f"xt{s}",
                            name=f"xt{s}") for s in range(EPC)]

            def mk_chunks(kind, s, bounds):
                tiles, lut = [], {}
                for c, (f0, f1) in enumerate(bounds):
                    t = wp.tile([P, f1 - f0, KO, P], mm_dt,
                                tag=f"{kind}{s}_{c}", name=f"{kind}{s}_{c}")
                    tiles.append(t)
                    for f in range(f0, f1):
                        lut[f] = (t, f - f0)
                return tiles, lut, bounds

            EV2 = [(0, 2), (2, 4), (4, 6), (6, 8)]
            EV4 = [(0, 4), (4, 8)]
            HEAD = [(0, 1), (1, 2), (2, 4), (4, 6), (6, 8)]
            wg_tiles = [mk_chunks("wg", 0, HEAD), mk_chunks("wg", 1, EV2)]
            wu_tiles = [mk_chunks("wu", 0, HEAD), mk_chunks("wu", 1, EV2)]
            wd_tiles = [mk_chunks("wd", s, EV2) for s in range(EPC)]

            def wchunk(eng, pack, dram, c):
                tiles, _, bounds = pack
                f0, f1 = bounds[c]
                eng.dma_start(
                    tiles[c],
                    dram[:, f0:f1]
                    .rearrange("p f (ko m) -> p f ko m", m=P))

            # ---- DMA issue ------------------------------------------
            # Exact consumption order, alternating the two HWDGE queues
            # item-by-item: both queues' cumulative delivery then tracks
            # consumption and neither in-order ring can block the PE.
            # Head: scalar is provably idle until the first activation
    	    # (~12.5us), and its first 3 DMA issues use fresh semaphores
            # (no reuse -> no engine-blocking wait).  Splitting the head
            # across both queues pulls the first matmul ~2.5us earlier.
            head_scalar = [("xt0b",), ("wu", 0, 0), ("wu", 0, 1)]
            issue_seq = [
                ("xt0a",),
                ("wg", 0, 0), ("wg", 0, 1), ("wg", 0, 2),
                ("wu", 0, 2),
                ("wg", 0, 3), ("wu", 0, 3),
                ("wd", 0, 0), ("wd", 0, 1),
                ("wg", 0, 4), ("wu", 0, 4),
                ("wd", 0, 2), ("wd", 0, 3),
                ("xt", 1),
                ("wg", 1, 0), ("wu", 1, 0), ("wg", 1, 1), ("wu", 1, 1),
                ("wg", 1, 2), ("wu", 1, 2),
                ("wd", 1, 0), ("wd", 1, 1),
                ("wg", 1, 3), ("wu", 1, 3),
                ("wd", 1, 2), ("wd", 1, 3),
            ]
            # All input DMAs ride the sync queue ONLY: Tile rotates ~9 DMA
            # semaphores, and on reuse the issue instruction BLOCKS the
            # issuing engine until the prior DMA on that semaphore lands.
            # Blocking sync is free (it has no other duties); blocking
            # scalar would starve the silu activations.  One queue
            # sustains ~400 B/ns across the 16 DMA engines.
            packs = {"wg": (wg_tiles, wg_d), "wu": (wu_tiles, wu_d),
                     "wd": (wd_tiles, wd_d)}

            def issue(eng, item):
                if item[0] == "xt0a":
                    eng.dma_start(xt_t[0][:, 0:KO // 2], xt_d[0][:, 0:KO // 2])
                elif item[0] == "xt0b":
                    eng.dma_start(xt_t[0][:, KO // 2:], xt_d[0][:, KO // 2:])
                elif item[0] == "xt":
                    eng.dma_start(xt_t[item[1]], xt_d[item[1]])
                else:
                    tiles, drams = packs[item[0]]
                    wchunk(eng, tiles[item[1]], drams[item[1]], item[2])

            for item in head_scalar:
                issue(nc.scalar, item)
            for item in issue_seq:
                issue(nc.sync, item)

            # ---- compute --------------------------------------------
            FSPLIT, YPRE = 6, 4  # y chains pre-started on first 6 f-blocks

            def gu_block(s, f, cap):
                wgt, fg = wg_tiles[s][1][f]
                wut, fu = wu_tiles[s][1][f]
                h_sb = h_t[s]
                psg = pp.tile([P, cap], f32, tag="ps")
                psu = pp.tile([P, cap], f32, tag="ps")
                for k in range(KO):
                    mm(psg, wgt[:, fg, k], xt_t[s][:, k], k == 0, k == KO - 1)
                for k in range(KO):
                    mm(psu, wut[:, fu, k], xt_t[s][:, k], k == 0, k == KO - 1)
                sg = sp.tile([P, cap], f32, tag="sg")
                nc.scalar.activation(sg, psg, silu)
                nc.vector.tensor_mul(out=h_sb[:, f], in0=sg, in1=psu)

            def y_out(s, hb, psy, cap):
                ysb = op.tile([P, cap], mm_dt, tag="y")
                nc.vector.tensor_copy(out=ysb, in_=psy)
                # slot 1's writes land after sync's weight stream has
                # drained; alternating queues halves the issue tail.
                eng = nc.sync if (s == 1 and hb % 2 == 0) else nc.scalar
                eng.dma_start(yt_d[s][hb], ysb)

            h_t = [hp.tile([P, FB, caps[s]], mm_dt, tag=f"h{s}",
                           name=f"h{s}") for s in range(EPC)]
            for s in range(EPC):
                cap = caps[s]
                assert cap <= 512
                h_sb = h_t[s]
                for f in range(FSPLIT):
                    gu_block(s, f, cap)
                # pre-start y chains on the first FSPLIT f-blocks while
                # the activation tail of f6/f7 drains behind the PE
                psy_live = []
                for hb in range(YPRE):
                    wdt, hl = wd_tiles[s][1][hb]
                    psy = pp.tile([P, cap], f32, tag="ps", name=f"psy{hb}")
                    for f in range(FSPLIT):
                        mm(psy, wdt[:, hl, f], h_sb[:, f], f == 0, False)
                    psy_live.append(psy)
                for f in range(FSPLIT, FB):
                    gu_block(s, f, cap)
                for hb in range(YPRE):
                    wdt, hl = wd_tiles[s][1][hb]
                    psy = psy_live[hb]
                    for f in range(FSPLIT, FB):
                        mm(psy, wdt[:, hl, f], h_sb[:, f], False, f == FB - 1)
                    y_out(s, hb, psy, cap)
                for hb in range(YPRE, HB):
                    wdt, hl = wd_tiles[s][1][hb]
                    psy = pp.tile([P, cap], f32, tag="ps")
                    for f in range(FB):
                        mm(psy, wdt[:, hl, f], h_sb[:, f], f == 0, f == FB - 1)
                    y_out(s, hb, psy, cap)

    nc.compile()
    return nc


def _route_np(x, gate_w, bias):
    """Numpy fallback router (same math, host BLAS numerics)."""
    gates = x.astype(np.float32) @ gate_w.T
    orig = 1.0 / (1.0 + np.exp(-gates))
    corrected = orig + bias
    inds = np.argsort(-corrected, axis=-1, kind="stable")[:, :KTOP].astype(np.int32)
    sel = np.take_along_axis(orig, inds, axis=-1)
    sel = sel / (sel.sum(axis=-1, keepdims=True) + 1e-20)
    return inds, sel.astype(np.float32)


def _route(x, gate_w, bias):
    """Top-2 routing with exactly the reference's jax ops on CPU."""
    try:
        import jax
        import jax.numpy as jnp
        cpu = jax.devices("cpu")[0]
    except Exception:
        return _route_np(x, gate_w, bias)
    with jax.default_device(cpu):
        xd = jax.device_put(x, cpu)
        gd = jax.device_put(gate_w, cpu)
        bd = jax.device_put(bias, cpu)
        gates = jnp.einsum("th,eh->te", xd.astype(jnp.float32), gd)
        orig = jax.nn.sigmoid(gates)
        corrected = orig + bd
        _, inds = jax.lax.top_k(corrected, KTOP)
        sel = jnp.take_along_axis(orig, inds, axis=-1)
        sel = sel / (jnp.sum(sel, axis=-1, keepdims=True) + 1e-20)
        sel = sel.astype(x.dtype)
    return np.asarray(inds), np.asarray(sel)


_PACK_CACHE = {}


NP_MM_DT = np.float16 if MM_MODE == "f16" else np.float32


def _pack(w):
    """[1024, 1024] -> [128, 8, 1024]: out[p, b, k*128+m] = w[k*128+p, b*128+m].

    Partition-major so a [p, f0:f1] DMA slice is one contiguous multi-KB
    run per partition (big DMA descriptors)."""
    return np.ascontiguousarray(
        w.reshape(8, P, 8, P).transpose(1, 2, 0, 3).reshape(P, 8, 8 * P)
        .astype(NP_MM_DT))


def kernel(x, gate_w, w_gate, w_up, w_down, e_score_correction_bias):
    global LAST_RESULTS
    from concourse import bass_utils

    x = np.asarray(x, dtype=np.float32)
    inds, sel = _route(x, np.asarray(gate_w, np.float32),
                       np.asarray(e_score_correction_bias, np.float32))

    # dispatch: token lists per expert
    tok_idx, tok_w = [], []
    for e in range(E):
        rows, slots = np.nonzero(inds == e)
        tok_idx.append(rows)
        tok_w.append(sel[rows, slots])
    counts = np.array([len(t) for t in tok_idx])

    # slot 0 of each core hosts one of the 8 largest experts, slot 1 one of
    # the 8 smallest; this minimizes cap0+cap1 = c[0]+c[8].
    order = np.argsort(-counts, kind="stable")
    assign = [(int(order[c]), int(order[E - 1 - c])) for c in range(NCORES)]

    def _cap(n):
        if MM_MODE == "f16":
            return max(64, -(-max(n, 1) // CAP_ALIGN) * CAP_ALIGN)
        return max(256, -(-max(n, 1) // 64) * 64)

    caps = tuple(_cap(int(counts[[assign[c][s] for c in range(NCORES)]].max()))
                 for s in range(EPC))

    nc = _build_program(caps)

    # weight packing (cached on the weight buffers' identity)
    wkey = (id(w_gate), id(w_up), id(w_down),
            w_gate.shape if hasattr(w_gate, "shape") else None)
    packed = _PACK_CACHE.get(wkey)
    if packed is None:
        wg = np.asarray(w_gate, np.float32)
        wu = np.asarray(w_up, np.float32)
        wd = np.asarray(w_down, np.float32)
        packed = ([_pack(wg[e]) for e in range(E)],
                  [_pack(wu[e]) for e in range(E)],
                  [_pack(wd[e]) for e in range(E)])
        _PACK_CACHE.clear()
        _PACK_CACHE[wkey] = packed
    wg_p, wu_p, wd_p = packed

    in_maps = []
    for c in range(NCORES):
        m = {}
        for s in range(EPC):
            e = assign[c][s]
            xt = np.zeros((P, KO, caps[s]), NP_MM_DT)
            cnt = len(tok_idx[e])
            if cnt:
                g = x[tok_idx[e]].astype(NP_MM_DT)  # [cnt, H]
                xt[:, :, :cnt] = g.reshape(cnt, KO, P).transpose(2, 1, 0)
            m[f"xt{s}"] = xt
            m[f"wg{s}"] = wg_p[e]
            m[f"wu{s}"] = wu_p[e]
            m[f"wd{s}"] = wd_p[e]
        in_maps.append(m)

    res = None
    last_err = None
    for attempt in range(3):
        try:
            res = bass_utils.run_bass_kernel_spmd(
                nc, in_maps, core_ids=list(range(NCORES)))
            break
        except Exception as err:  # transient NRT/device errors happen
            last_err = err
            import time as _time
            _time.sleep(3.0 * (attempt + 1))
    if res is None:
        raise last_err
    LAST_RESULTS = res

    y = np.zeros((x.shape[0], H), np.float32)
    for c in range(NCORES):
        for s in range(EPC):
            e = assign[c][s]
            cnt = len(tok_idx[e])
            if not cnt:
                continue
            yt = res.results[c][f"yt{s}"].reshape(H, caps[s])
            y[tok_idx[e]] += (tok_w[e][:, None]
                              * yt[:, :cnt].T.astype(np.float32))
    return y


# revision 31
# speedup vs baseline: 1.0306x; 1.0306x over previous
"""MiniMax sparse-MoE block on 8 Trainium2 NeuronCores.

Strategy (expert-parallel, per the sharding hint):
  - Router (gates matmul + sigmoid + top-2 + weight normalization) runs on
    host CPU with exactly the reference's jax ops, bit-matching its
    routing decisions.  This *is* the dispatch step: tokens are gathered
    per selected expert while building the per-core input shards.
  - Each of the 8 cores owns E/8 = 2 experts.  A core runs the SwitchGLU
    MLP (silu(x@w_gate) * (x@w_up)) @ w_down for the tokens routed to its
    experts only (capacity = max expert load over cores per slot), with
    weights stationary on the PE array and tokens as the moving operand
    (activations kept transposed: [H, tokens]).
  - Matmuls run in fp16 (half the HBM traffic of fp32, full-rate PE);
    PSUM accumulation is fp32.
  - Schedule: ALL weight tiles are SBUF-resident (12.6 MB of 24 MB), so
    every weight DMA is issued up-front in consumption-priority order on
    the two HWDGE queues (sync + scalar) with small leading chunks; the
    PE never waits on buffer reuse and starts ~2 us into the body.
  - y is written back as fp16 (halves the writeback bytes); the host
    combines y[t] = sum over the token's 2 experts of sel * expert_out in
    fp32.
"""

import os
import sys
import functools

for _p in ("/opt/trn_rl_repo", "/root/.axon_site/_ro/trn_rl_repo"):
    if os.path.isdir(_p) and _p not in sys.path:
        sys.path.append(_p)

import numpy as np

T, H, F, E, KTOP = 2048, 1024, 1024, 16, 2
NCORES = 8
EPC = E // NCORES  # experts per core
P = 128
KO = H // P  # contraction chunks per 1024-dim
FB = F // P  # 128-blocks of F
HB = H // P  # 128-blocks of H

MM_MODE = os.environ.get("MOE_MM_MODE", "f16")
CAP_ALIGN = int(os.environ.get("MOE_CAP_ALIGN", "16"))

LAST_RESULTS = None  # BassKernelResults of the most recent device run


def _chunks(cap):
    """Split cap into moving-dim chunks <= 512 (PSUM bank fp32 limit)."""
    out, rem, n = [], cap, -(-cap // 512)
    for i in range(n):
        c = min(512, rem, -(-rem // ((n - i) * 64)) * 64)
        out.append(c)
        rem -= c
    assert sum(out) == cap and all(0 < c <= 512 for c in out), (cap, out)
    return out


@functools.lru_cache(maxsize=4)
def _build_program(caps):
    import concourse.mybir as mybir
    import concourse.tile as tile
    from concourse import bacc

    f32 = mybir.dt.float32
    f16 = mybir.dt.float16
    mm_dt = {"f16": f16, "f32r": mybir.dt.float32r, "f32": f32}[MM_MODE]
    silu = mybir.ActivationFunctionType.Silu

    nc = bacc.Bacc("TRN2", target_bir_lowering=False, debug=False,
                   num_devices=NCORES)

    xt_d, wg_d, wu_d, wd_d, yt_d = [], [], [], [], []
    for s in range(EPC):
        cap = caps[s]
        xt_d.append(nc.dram_tensor(f"xt{s}", [P, KO, cap], mm_dt,
                                   kind="ExternalInput").ap())
        wg_d.append(nc.dram_tensor(f"wg{s}", [P, FB, H], mm_dt,
                                   kind="ExternalInput").ap())
        wu_d.append(nc.dram_tensor(f"wu{s}", [P, FB, H], mm_dt,
                                   kind="ExternalInput").ap())
        wd_d.append(nc.dram_tensor(f"wd{s}", [P, HB, F], mm_dt,
                                   kind="ExternalInput").ap())
        yt_d.append(nc.dram_tensor(f"yt{s}", [HB, P, cap], mm_dt,
                                   kind="ExternalOutput").ap())

    def mm(ps, lhsT, rhs, start, stop):
        nc.tensor.matmul(ps, lhsT=lhsT, rhs=rhs, start=start, stop=stop)

    with tile.TileContext(nc) as tc:
        with (
            tc.tile_pool(name="xp", bufs=1) as xp,
            tc.tile_pool(name="wp", bufs=1) as wp,
            tc.tile_pool(name="sp", bufs=3) as sp,
            tc.tile_pool(name="hp", bufs=1) as hp,
            tc.tile_pool(name="op", bufs=8) as op,
            tc.tile_pool(name="pp", bufs=8, space="PSUM") as pp,
        ):
            # ---- resident tiles, one per DMA chunk ------------------
            # Distinct tags per chunk keep Tile's DMA->matmul deps
            # fine-grained.  Fine (2-f-block) chunks where the PE is
            # close on the stream's heels (slot-0 g/u); coarse (4-f)
            # chunks later, to bound the semaphore count (the exit
            # epilogue clears each semaphore individually).
            xt_t = [xp.tile([P, KO, caps[s]], mm_dt, tag=f"xt{s}",
                            name=f"xt{s}") for s in range(EPC)]

            def mk_chunks(kind, s, bounds):
                tiles, lut = [], {}
                for c, (f0, f1) in enumerate(bounds):
                    t = wp.tile([P, f1 - f0, KO, P], mm_dt,
                                tag=f"{kind}{s}_{c}", name=f"{kind}{s}_{c}")
                    tiles.append(t)
                    for f in range(f0, f1):
                        lut[f] = (t, f - f0)
                return tiles, lut, bounds

            EV2 = [(0, 2), (2, 4), (4, 6), (6, 8)]
            EV4 = [(0, 4), (4, 8)]
            HEAD = [(0, 1), (1, 2), (2, 3), (3, 4), (4, 5), (5, 6), (6, 8)]
            wg_tiles = [mk_chunks("wg", 0, HEAD), mk_chunks("wg", 1, EV2)]
            wu_tiles = [mk_chunks("wu", 0, HEAD), mk_chunks("wu", 1, EV2)]
            wd_tiles = [mk_chunks("wd", s, EV2) for s in range(EPC)]

            def wchunk(eng, pack, dram, c):
                tiles, _, bounds = pack
                f0, f1 = bounds[c]
                eng.dma_start(
                    tiles[c],
                    dram[:, f0:f1]
                    .rearrange("p f (ko m) -> p f ko m", m=P))

            # ---- DMA issue ------------------------------------------
            # Exact consumption order, alternating the two HWDGE queues
            # item-by-item: both queues' cumulative delivery then tracks
            # consumption and neither in-order ring can block the PE.
            # Head: scalar is provably idle until the first activation
    	    # (~12.5us), and its first 3 DMA issues use fresh semaphores
            # (no reuse -> no engine-blocking wait).  Splitting the head
            # across both queues pulls the first matmul ~2.5us earlier.
            head_scalar = [("xt0b",), ("wu", 0, 0), ("wu", 0, 1)]
            issue_seq = [
                ("xt0a",),
                ("wg", 0, 0), ("wg", 0, 1), ("wg", 0, 2),
                ("wu", 0, 2), ("wg", 0, 3), ("wu", 0, 3),
                ("wg", 0, 4), ("wu", 0, 4),
                ("wg", 0, 5), ("wu", 0, 5),
                ("wd", 0, 0), ("wd", 0, 1),
                ("wg", 0, 6), ("wu", 0, 6),
                ("wd", 0, 2), ("wd", 0, 3),
                ("xt", 1),
                ("wg", 1, 0), ("wu", 1, 0), ("wg", 1, 1), ("wu", 1, 1),
                ("wg", 1, 2), ("wu", 1, 2),
                ("wd", 1, 0), ("wd", 1, 1),
                ("wg", 1, 3), ("wu", 1, 3),
                ("wd", 1, 2), ("wd", 1, 3),
            ]
            # All input DMAs ride the sync queue ONLY: Tile rotates ~9 DMA
            # semaphores, and on reuse the issue instruction BLOCKS the
            # issuing engine until the prior DMA on that semaphore lands.
            # Blocking sync is free (it has no other duties); blocking
            # scalar would starve the silu activations.  One queue
            # sustains ~400 B/ns across the 16 DMA engines.
            packs = {"wg": (wg_tiles, wg_d), "wu": (wu_tiles, wu_d),
                     "wd": (wd_tiles, wd_d)}

            def issue(eng, item):
                if item[0] == "xt0a":
                    eng.dma_start(xt_t[0][:, 0:KO // 2], xt_d[0][:, 0:KO // 2])
                elif item[0] == "xt0b":
                    eng.dma_start(xt_t[0][:, KO // 2:], xt_d[0][:, KO // 2:])
                elif item[0] == "xt":
                    eng.dma_start(xt_t[item[1]], xt_d[item[1]])
                else:
                    tiles, drams = packs[item[0]]
                    wchunk(eng, tiles[item[1]], drams[item[1]], item[2])

            for item in head_scalar:
                issue(nc.scalar, item)
            for item in issue_seq:
                issue(nc.sync, item)

            # ---- compute --------------------------------------------
            FSPLIT, YPRE = 6, 4  # y chains pre-started on first 6 f-blocks

            def gu_block(s, f, cap):
                wgt, fg = wg_tiles[s][1][f]
                wut, fu = wu_tiles[s][1][f]
                h_sb = h_t[s]
                psg = pp.tile([P, cap], f32, tag="ps")
                psu = pp.tile([P, cap], f32, tag="ps")
                for k in range(KO):
                    mm(psg, wgt[:, fg, k], xt_t[s][:, k], k == 0, k == KO - 1)
                for k in range(KO):
                    mm(psu, wut[:, fu, k], xt_t[s][:, k], k == 0, k == KO - 1)
                sg = sp.tile([P, cap], f32, tag="sg")
                nc.scalar.activation(sg, psg, silu)
                nc.vector.tensor_mul(out=h_sb[:, f], in0=sg, in1=psu)

            def y_out(s, hb, psy, cap):
                ysb = op.tile([P, cap], mm_dt, tag="y")
                nc.vector.tensor_copy(out=ysb, in_=psy)
                # slot 1's writes land after sync's weight stream has
                # drained; alternating queues halves the issue tail.
                eng = nc.sync if (s == 1 and hb % 2 == 0) else nc.scalar
                eng.dma_start(yt_d[s][hb], ysb)

            h_t = [hp.tile([P, FB, caps[s]], mm_dt, tag=f"h{s}",
                           name=f"h{s}") for s in range(EPC)]
            for s in range(EPC):
                cap = caps[s]
                assert cap <= 512
                h_sb = h_t[s]
                for f in range(FSPLIT):
                    gu_block(s, f, cap)
                # pre-start y chains on the first FSPLIT f-blocks while
                # the activation tail of f6/f7 drains behind the PE
                psy_live = []
                for hb in range(YPRE):
                    wdt, hl = wd_tiles[s][1][hb]
                    psy = pp.tile([P, cap], f32, tag="ps", name=f"psy{hb}")
                    for f in range(FSPLIT):
                        mm(psy, wdt[:, hl, f], h_sb[:, f], f == 0, False)
                    psy_live.append(psy)
                for f in range(FSPLIT, FB):
                    gu_block(s, f, cap)
                for hb in range(YPRE):
                    wdt, hl = wd_tiles[s][1][hb]
                    psy = psy_live[hb]
                    for f in range(FSPLIT, FB):
                        mm(psy, wdt[:, hl, f], h_sb[:, f], False, f == FB - 1)
                    y_out(s, hb, psy, cap)
                for hb in range(YPRE, HB):
                    wdt, hl = wd_tiles[s][1][hb]
                    psy = pp.tile([P, cap], f32, tag="ps")
                    for f in range(FB):
                        mm(psy, wdt[:, hl, f], h_sb[:, f], f == 0, f == FB - 1)
                    y_out(s, hb, psy, cap)

    nc.compile()
    return nc


def _route_np(x, gate_w, bias):
    """Numpy fallback router (same math, host BLAS numerics)."""
    gates = x.astype(np.float32) @ gate_w.T
    orig = 1.0 / (1.0 + np.exp(-gates))
    corrected = orig + bias
    inds = np.argsort(-corrected, axis=-1, kind="stable")[:, :KTOP].astype(np.int32)
    sel = np.take_along_axis(orig, inds, axis=-1)
    sel = sel / (sel.sum(axis=-1, keepdims=True) + 1e-20)
    return inds, sel.astype(np.float32)


def _route(x, gate_w, bias):
    """Top-2 routing with exactly the reference's jax ops on CPU."""
    try:
        import jax
        import jax.numpy as jnp
        cpu = jax.devices("cpu")[0]
    except Exception:
        return _route_np(x, gate_w, bias)
    with jax.default_device(cpu):
        xd = jax.device_put(x, cpu)
        gd = jax.device_put(gate_w, cpu)
        bd = jax.device_put(bias, cpu)
        gates = jnp.einsum("th,eh->te", xd.astype(jnp.float32), gd)
        orig = jax.nn.sigmoid(gates)
        corrected = orig + bd
        _, inds = jax.lax.top_k(corrected, KTOP)
        sel = jnp.take_along_axis(orig, inds, axis=-1)
        sel = sel / (jnp.sum(sel, axis=-1, keepdims=True) + 1e-20)
        sel = sel.astype(x.dtype)
    return np.asarray(inds), np.asarray(sel)


_PACK_CACHE = {}


NP_MM_DT = np.float16 if MM_MODE == "f16" else np.float32


def _pack(w):
    """[1024, 1024] -> [128, 8, 1024]: out[p, b, k*128+m] = w[k*128+p, b*128+m].

    Partition-major so a [p, f0:f1] DMA slice is one contiguous multi-KB
    run per partition (big DMA descriptors)."""
    return np.ascontiguousarray(
        w.reshape(8, P, 8, P).transpose(1, 2, 0, 3).reshape(P, 8, 8 * P)
        .astype(NP_MM_DT))


def kernel(x, gate_w, w_gate, w_up, w_down, e_score_correction_bias):
    global LAST_RESULTS
    from concourse import bass_utils

    x = np.asarray(x, dtype=np.float32)
    inds, sel = _route(x, np.asarray(gate_w, np.float32),
                       np.asarray(e_score_correction_bias, np.float32))

    # dispatch: token lists per expert
    tok_idx, tok_w = [], []
    for e in range(E):
        rows, slots = np.nonzero(inds == e)
        tok_idx.append(rows)
        tok_w.append(sel[rows, slots])
    counts = np.array([len(t) for t in tok_idx])

    # slot 0 of each core hosts one of the 8 largest experts, slot 1 one of
    # the 8 smallest; this minimizes cap0+cap1 = c[0]+c[8].
    order = np.argsort(-counts, kind="stable")
    assign = [(int(order[c]), int(order[E - 1 - c])) for c in range(NCORES)]

    def _cap(n):
        if MM_MODE == "f16":
            return max(64, -(-max(n, 1) // CAP_ALIGN) * CAP_ALIGN)
        return max(256, -(-max(n, 1) // 64) * 64)

    caps = tuple(_cap(int(counts[[assign[c][s] for c in range(NCORES)]].max()))
                 for s in range(EPC))

    nc = _build_program(caps)

    # weight packing (cached on the weight buffers' identity)
    wkey = (id(w_gate), id(w_up), id(w_down),
            w_gate.shape if hasattr(w_gate, "shape") else None)
    packed = _PACK_CACHE.get(wkey)
    if packed is None:
        wg = np.asarray(w_gate, np.float32)
        wu = np.asarray(w_up, np.float32)
        wd = np.asarray(w_down, np.float32)
        packed = ([_pack(wg[e]) for e in range(E)],
                  [_pack(wu[e]) for e in range(E)],
                  [_pack(wd[e]) for e in range(E)])
        _PACK_CACHE.clear()
        _PACK_CACHE[wkey] = packed
    wg_p, wu_p, wd_p = packed

    in_maps = []
    for c in range(NCORES):
        m = {}
        for s in range(EPC):
            e = assign[c][s]
            xt = np.zeros((P, KO, caps[s]), NP_MM_DT)
            cnt = len(tok_idx[e])
            if cnt:
                g = x[tok_idx[e]].astype(NP_MM_DT)  # [cnt, H]
                xt[:, :, :cnt] = g.reshape(cnt, KO, P).transpose(2, 1, 0)
            m[f"xt{s}"] = xt
            m[f"wg{s}"] = wg_p[e]
            m[f"wu{s}"] = wu_p[e]
            m[f"wd{s}"] = wd_p[e]
        in_maps.append(m)

    res = None
    last_err = None
    for attempt in range(3):
        try:
            res = bass_utils.run_bass_kernel_spmd(
                nc, in_maps, core_ids=list(range(NCORES)))
            break
        except Exception as err:  # transient NRT/device errors happen
            last_err = err
            import time as _time
            _time.sleep(3.0 * (attempt + 1))
    if res is None:
        raise last_err
    LAST_RESULTS = res

    y = np.zeros((x.shape[0], H), np.float32)
    for c in range(NCORES):
        for s in range(EPC):
            e = assign[c][s]
            cnt = len(tok_idx[e])
            if not cnt:
                continue
            yt = res.results[c][f"yt{s}"].reshape(H, caps[s])
            y[tok_idx[e]] += (tok_w[e][:, None]
                              * yt[:, :cnt].T.astype(np.float32))
    return y
